# revision 11
# baseline (speedup 1.0000x reference)
"""Trainium2 Bass kernel for nn_CausalGNN (segment_reduce), self-contained.

Sharding: data-parallel over the 4096 graphs -- core m owns graphs
[512m, 512(m+1)) and every h_sub row whose parent graph lies in that
range.  The host stable-sorts rows by parent graph, splits each core's
512 graphs into 4 chunks of 128 segments, and pads each chunk to a
whole number of 512-row blocks so all 8 cores run one SPMD program.

Per core the device computes:
  * mask-MLP layer 1 as f32r (fp22) matmuls: z' = x @ (W1 * |W2|) with
    columns sign-sorted (positives first).  Inputs are pre-rounded RNE
    to fp22 on host so the PE truncation is lossless.
  * layer 2 collapses into two Relu+row-sum activations (ACT accum_out):
    s = sum(relu(z'[:, :PPOS])) - sum(relu(z'[:, PPOS:])), which lands
    directly in [row, 1] orientation for thresholding.
  * valid/env one-hot matrices (seg-id equality vs an iota constant,
    masked by the threshold flags) feed bf16 matmuls that accumulate
    [128 segs, 256 dims | count] sums in PSUM across each chunk.
  * aligned/env means, and the final head out = [h_graph|aligned] @ Wc.

The B x B contrastive term collapses algebraically to O(B*D):
  sum_j (1 - cos(a_i, b_j)) = #j - ahat_i . sum_j bhat_j,
so the host only gathers the per-graph [B, D] means and finishes the
loss with two dot products per row.
"""

import math
from contextlib import ExitStack

import numpy as np
import ml_dtypes

import concourse.bass as bass
import concourse.bacc as bacc
import concourse.mybir as mybir
from concourse.alu_op_type import AluOpType
from concourse.tile import TileContext

P = 128
NCORES = 8
EPS = 1e-8
THR_HI = 0.4
THR_LO = 0.3  # THRESHOLD - 0.1

F32 = mybir.dt.float32
F32R = mybir.dt.float32r
BF16 = mybir.dt.bfloat16


def _rne_fp22(x: np.ndarray) -> np.ndarray:
    """Round fp32 to nearest-even fp22 (e8m13): the PE reads f32r operands
    by truncating to fp22, so pre-rounded values pass through losslessly."""
    xi = np.ascontiguousarray(x, dtype=np.float32).view(np.uint32).astype(np.uint64)
    low = xi & 0x3FF
    base = xi & ~np.uint64(0x3FF)
    up = (low > 0x200) | ((low == 0x200) & (((xi >> 10) & 1) == 1))
    out = (base + (up.astype(np.uint64) << 10)).astype(np.uint32)
    return out.view(np.float32).reshape(x.shape)


def _logit(p: float) -> float:
    return math.log(p / (1.0 - p))


def _build_program(NPAD, CB, PPOS, D, H, T, b2f, has_b1, has_bc):
    """One SPMD NeuronCore program; all 8 cores run it on their shard."""
    SEG = 512          # graphs per core
    NBLK = 4 * CB      # 512-row blocks per core
    NT = NPAD // P     # 128-row tiles per core
    KD = D // P        # k-chunks of the feature dim (2)

    nc = bacc.Bacc("TRN2", target_bir_lowering=False, debug=False, num_devices=NCORES)

    xt_d = nc.dram_tensor("xt", [D, NPAD], F32, kind="ExternalInput")
    xa_d = nc.dram_tensor("xa", [NPAD, D + 2], BF16, kind="ExternalInput")
    sg_d = nc.dram_tensor("sg", [NT, P, 1], F32, kind="ExternalInput")
    w1_d = nc.dram_tensor("w1p", [D + 1, H], F32, kind="ExternalInput")
    hgt_d = nc.dram_tensor("hgt", [D, SEG], F32, kind="ExternalInput")
    wc_d = nc.dram_tensor("wc", [2 * D, T], F32, kind="ExternalInput")
    iota_d = nc.dram_tensor("iota", [P, P], F32, kind="ExternalInput")
    ident_d = nc.dram_tensor("ident", [P, P], F32, kind="ExternalInput")
    if has_bc:
        bcb_d = nc.dram_tensor("bcb", [P, T], F32, kind="ExternalInput")

    mask_d = nc.dram_tensor("mask_o", [NT, P], F32, kind="ExternalOutput")
    al_d = nc.dram_tensor("al_o", [SEG, D], F32, kind="ExternalOutput")
    ev_d = nc.dram_tensor("ev_o", [SEG, D], F32, kind="ExternalOutput")
    cnt_d = nc.dram_tensor("cnt_o", [SEG, 2], F32, kind="ExternalOutput")
    out_d = nc.dram_tensor("out_o", [SEG, T], F32, kind="ExternalOutput")

    thr_hi = _logit(THR_HI) - b2f
    thr_lo = _logit(THR_LO) - b2f

    with TileContext(nc) as tc, ExitStack() as ctx:
        cpool = ctx.enter_context(tc.tile_pool(name="consts", bufs=1))
        xtp = ctx.enter_context(tc.tile_pool(name="xtp", bufs=3))
        xap = ctx.enter_context(tc.tile_pool(name="xap", bufs=8))
        sgp = ctx.enter_context(tc.tile_pool(name="sgp", bufs=8))
        ohp = ctx.enter_context(tc.tile_pool(name="ohp", bufs=6))
        smp = ctx.enter_context(tc.tile_pool(name="smp", bufs=8))
        scp = ctx.enter_context(tc.tile_pool(name="scp", bufs=2))
        alp = ctx.enter_context(tc.tile_pool(name="alp", bufs=2))
        zpp = ctx.enter_context(tc.tile_pool(name="zpp", bufs=2, space="PSUM"))
        svp = ctx.enter_context(tc.tile_pool(name="svp", bufs=1, space="PSUM"))
        tpp = ctx.enter_context(tc.tile_pool(name="tpp", bufs=1, space="PSUM"))
        opp = ctx.enter_context(tc.tile_pool(name="opp", bufs=1, space="PSUM"))

        # --- constants ---
        w1a = cpool.tile([P, H], F32)
        nc.sync.dma_start(out=w1a[:], in_=w1_d[0:P, :])
        w1b = cpool.tile([P, H], F32)
        nc.sync.dma_start(out=w1b[:], in_=w1_d[P : 2 * P, :])
        if has_b1:
            w1bias = cpool.tile([1, H], F32)
            nc.sync.dma_start(out=w1bias[:], in_=w1_d[2 * P : 2 * P + 1, :])
            ones_row = cpool.tile([1, P], F32)
            nc.vector.memset(ones_row[:], 1.0)
        hgt0 = cpool.tile([P, SEG], F32)
        nc.sync.dma_start(out=hgt0[:], in_=hgt_d[0:P, :])
        hgt1 = cpool.tile([P, SEG], F32)
        nc.sync.dma_start(out=hgt1[:], in_=hgt_d[P : 2 * P, :])
        wc_sb = cpool.tile([P, 4 * T], F32)
        for k in range(4):
            nc.sync.dma_start(
                out=wc_sb[:, k * T : (k + 1) * T], in_=wc_d[k * P : (k + 1) * P, :]
            )
        iota_sb = cpool.tile([P, P], F32)
        nc.sync.dma_start(out=iota_sb[:], in_=iota_d[:, :])
        ident_sb = cpool.tile([P, P], F32)
        nc.sync.dma_start(out=ident_sb[:], in_=ident_d[:, :])
        if has_bc:
            bcb_sb = cpool.tile([P, T], F32)
            nc.sync.dma_start(out=bcb_sb[:], in_=bcb_d[:, :])
        b2_sb = cpool.tile([P, 1], F32)
        nc.vector.memset(b2_sb[:], b2f)
        mask_sb = cpool.tile([P, NT], F32)

        for c in range(4):  # 128-segment chunks
            sv = svp.tile([P, D + 2], F32, tag="sv")
            se = svp.tile([P, D + 2], F32, tag="se")
            for b in range(CB):
                blk = c * CB + b
                xt0 = xtp.tile([P, 512], F32, tag="xt0")
                nc.sync.dma_start(out=xt0[:], in_=xt_d[0:P, blk * 512 : (blk + 1) * 512])
                xt1 = xtp.tile([P, 512], F32, tag="xt1")
                nc.sync.dma_start(out=xt1[:], in_=xt_d[P : 2 * P, blk * 512 : (blk + 1) * 512])
                for rc in range(4):
                    t = blk * 4 + rc
                    first = b == 0 and rc == 0
                    last = b == CB - 1 and rc == 3

                    xa_t = xap.tile([P, D + 2], BF16, tag="xa")
                    nc.sync.dma_start(out=xa_t[:], in_=xa_d[t * P : (t + 1) * P, :])
                    sg_t = sgp.tile([P, 1], F32, tag="sg")
                    nc.sync.dma_start(out=sg_t[:], in_=sg_d[t])

                    # mask MLP layer 1: z = x[tile] @ W1'   -> PSUM [128, H]
                    z = zpp.tile([P, H], F32, tag="z")
                    rsl = slice(rc * P, (rc + 1) * P)
                    nc.tensor.matmul(
                        z[:], xt0[:, rsl], w1a[:],
                        start=True, stop=False,
                    )
                    nc.tensor.matmul(
                        z[:], xt1[:, rsl], w1b[:],
                        start=False, stop=not has_b1,
                    )
                    if has_b1:
                        nc.tensor.matmul(
                            z[:], ones_row[:], w1bias[:],
                            start=False, stop=True,
                        )

                    # layer 2: s = sum(relu(z_pos)) - sum(relu(z_neg))
                    scratch = scp.tile([P, H], F32, tag="scr")
                    s_pos = smp.tile([P, 1], F32, tag="sp")
                    s_neg = smp.tile([P, 1], F32, tag="sn")
                    if PPOS > 0:
                        nc.scalar.activation(
                            out=scratch[:, :PPOS], in_=z[:, :PPOS],
                            func=mybir.ActivationFunctionType.Relu, accum_out=s_pos[:],
                        )
                    else:
                        nc.vector.memset(s_pos[:], 0.0)
                    if PPOS < H:
                        nc.scalar.activation(
                            out=scratch[:, PPOS:], in_=z[:, PPOS:],
                            func=mybir.ActivationFunctionType.Relu, accum_out=s_neg[:],
                        )
                    else:
                        nc.vector.memset(s_neg[:], 0.0)
                    s_net = smp.tile([P, 1], F32, tag="snet")
                    nc.vector.tensor_tensor(
                        out=s_net[:], in0=s_pos[:], in1=s_neg[:], op=AluOpType.subtract
                    )

                    # graded mask output: sigmoid(s + b2)
                    nc.scalar.activation(
                        out=mask_sb[:, t : t + 1], in_=s_net[:],
                        func=mybir.ActivationFunctionType.Sigmoid, bias=b2_sb[:, 0:1],
                    )

                    # threshold flags (s-space)
                    valid = smp.tile([P, 1], F32, tag="vld")
                    nc.vector.tensor_scalar(
                        out=valid[:], in0=s_net[:], scalar1=thr_hi, scalar2=None,
                        op0=AluOpType.is_gt,
                    )
                    envm = smp.tile([P, 1], F32, tag="env")
                    nc.vector.tensor_scalar(
                        out=envm[:], in0=s_net[:], scalar1=thr_lo, scalar2=None,
                        op0=AluOpType.is_le,
                    )

                    # one-hot [row, seg] and masked variants
                    oh_eq = ohp.tile([P, P], BF16, tag="oheq")
                    nc.vector.tensor_scalar(
                        out=oh_eq[:], in0=iota_sb[:], scalar1=sg_t[:], scalar2=None,
                        op0=AluOpType.is_equal,
                    )
                    oh_v = ohp.tile([P, P], BF16, tag="ohv")
                    nc.vector.tensor_scalar(
                        out=oh_v[:], in0=oh_eq[:], scalar1=valid[:], scalar2=None,
                        op0=AluOpType.mult,
                    )
                    oh_e = ohp.tile([P, P], BF16, tag="ohe")
                    nc.vector.tensor_scalar(
                        out=oh_e[:], in0=oh_eq[:], scalar1=envm[:], scalar2=None,
                        op0=AluOpType.mult,
                    )

                    # segment sums: [seg, d | count] += onehot^T @ [x | 1]
                    nc.tensor.matmul(
                        sv[:], oh_v[:], xa_t[:], start=first, stop=last,
                        skip_group_check=True,
                    )
                    nc.tensor.matmul(
                        se[:], oh_e[:], xa_t[:], start=first, stop=last,
                        skip_group_check=True,
                    )

            # --- chunk epilogue: means, outputs, head matmul ---
            gsl = slice(c * P, (c + 1) * P)
            cnts = alp.tile([P, 2], F32, tag="cnts")
            nc.vector.tensor_copy(out=cnts[:, 0:1], in_=sv[:, D : D + 1])
            nc.vector.tensor_copy(out=cnts[:, 1:2], in_=se[:, D : D + 1])
            nc.sync.dma_start(out=cnt_d[gsl, :], in_=cnts[:])

            mx_v = smp.tile([P, 1], F32, tag="mxv")
            nc.vector.tensor_scalar(
                out=mx_v[:], in0=sv[:, D : D + 1], scalar1=1.0, scalar2=None,
                op0=AluOpType.max,
            )
            rc_v = smp.tile([P, 1], F32, tag="rcv")
            nc.vector.reciprocal(out=rc_v[:], in_=mx_v[:])
            al_c = alp.tile([P, D], F32, tag="al")
            nc.vector.tensor_scalar(
                out=al_c[:], in0=sv[:, 0:D], scalar1=rc_v[:], scalar2=None,
                op0=AluOpType.mult,
            )
            nc.sync.dma_start(out=al_d[gsl, :], in_=al_c[:])

            mx_e = smp.tile([P, 1], F32, tag="mxe")
            nc.vector.tensor_scalar(
                out=mx_e[:], in0=se[:, D : D + 1], scalar1=1.0, scalar2=None,
                op0=AluOpType.max,
            )
            rc_e = smp.tile([P, 1], F32, tag="rce")
            nc.vector.reciprocal(out=rc_e[:], in_=mx_e[:])
            ev_c = alp.tile([P, D], F32, tag="ev")
            nc.vector.tensor_scalar(
                out=ev_c[:], in0=se[:, 0:D], scalar1=rc_e[:], scalar2=None,
                op0=AluOpType.mult,
            )
            nc.sync.dma_start(out=ev_d[gsl, :], in_=ev_c[:])

            # aligned^T for the head matmul (PE transpose via identity)
            alT0 = alp.tile([P, P], F32, tag="alT0")
            alT1 = alp.tile([P, P], F32, tag="alT1")
            for half, alT in ((0, alT0), (1, alT1)):
                tp = tpp.tile([P, P], F32, tag="tp")
                nc.tensor.transpose(tp[:], al_c[:, half * P : (half + 1) * P], ident_sb[:])
                nc.vector.tensor_copy(out=alT[:], in_=tp[:])

            op_ps = opp.tile([P, T], F32, tag="op")
            nc.tensor.matmul(op_ps[:], hgt0[:, gsl], wc_sb[:, 0:T], start=True, stop=False)
            nc.tensor.matmul(op_ps[:], hgt1[:, gsl], wc_sb[:, T : 2 * T], start=False, stop=False)
            nc.tensor.matmul(op_ps[:], alT0[:], wc_sb[:, 2 * T : 3 * T], start=False, stop=False)
            nc.tensor.matmul(op_ps[:], alT1[:], wc_sb[:, 3 * T : 4 * T], start=False, stop=True)
            out_sb = alp.tile([P, T], F32, tag="outsb")
            if has_bc:
                nc.vector.tensor_tensor(
                    out=out_sb[:], in0=op_ps[:], in1=bcb_sb[:], op=AluOpType.add
                )
            else:
                nc.vector.tensor_copy(out=out_sb[:], in_=op_ps[:])
            nc.sync.dma_start(out=out_d[gsl, :], in_=out_sb[:])

        nc.sync.dma_start(out=mask_d.rearrange("t p -> p t"), in_=mask_sb[:])

    nc.finalize()
    return nc


def _prepare(inputs):
    """Host-side sharding/packing.  Returns (meta, in_maps)."""
    h_graph = np.ascontiguousarray(np.asarray(inputs["h_graph"], dtype=np.float32))
    h_sub = np.ascontiguousarray(np.asarray(inputs["h_sub"], dtype=np.float32))
    W1 = np.asarray(inputs["W1"], dtype=np.float32)
    b1 = np.asarray(inputs["b1"], dtype=np.float32)
    W2 = np.asarray(inputs["W2"], dtype=np.float32)
    b2 = np.asarray(inputs["b2"], dtype=np.float32)
    Wc = np.asarray(inputs["Wc"], dtype=np.float32)
    bc = np.asarray(inputs["bc"], dtype=np.float32)
    s2g = np.asarray(inputs["sub2graph"], dtype=np.int32)

    B, D = h_graph.shape
    N = h_sub.shape[0]
    H = W1.shape[1]
    T = Wc.shape[1]
    SEG = B // NCORES
    assert SEG == 512 and D == 256, (B, D)

    order = np.argsort(s2g, kind="stable")
    cnts_g = np.bincount(s2g, minlength=B)
    cnts_chunk = cnts_g.reshape(NCORES * 4, P).sum(1)  # rows per 128-graph chunk
    CB = max(1, int(math.ceil(cnts_chunk.max() / 512.0)))
    CHROWS = CB * 512
    NPAD = 4 * CHROWS
    NT = NPAD // P

    chunk_starts = np.zeros(NCORES * 4, dtype=np.int64)
    chunk_starts[1:] = np.cumsum(cnts_chunk)[:-1]

    slot_orig = np.full((NCORES, NPAD), -1, dtype=np.int64)
    for m in range(NCORES):
        for c in range(4):
            gc = m * 4 + c
            n_mc = int(cnts_chunk[gc])
            rows = order[chunk_starts[gc] : chunk_starts[gc] + n_mc]
            slot_orig[m, c * CHROWS : c * CHROWS + n_mc] = rows

    # fold |W2| into W1 and sign-sort columns (positives first)
    w2v = W2[:, 0]
    jperm = np.argsort(w2v < 0, kind="stable")
    PPOS = int((w2v >= 0).sum())
    W1p = (W1 * np.abs(w2v)[None, :])[:, jperm]
    b1p = (b1 * np.abs(w2v))[jperm]
    w1p_full = np.concatenate([W1p, b1p[None, :]], axis=0).astype(np.float32)
    has_b1 = bool(np.any(b1))
    has_bc = bool(np.any(bc))
    b2f = float(b2[0])

    iota_np = np.tile(np.arange(P, dtype=np.float32), (P, 1))
    ident_np = np.eye(P, dtype=np.float32)
    wc_np = np.ascontiguousarray(Wc, dtype=np.float32)
    bcb_np = np.tile(bc[None, :], (P, 1)).astype(np.float32) if has_bc else None

    in_maps = []
    for m in range(NCORES):
        sel = slot_orig[m]
        vmask = sel >= 0
        rows = np.zeros((NPAD, D), dtype=np.float32)
        rows[vmask] = h_sub[sel[vmask]]

        xa = np.zeros((NPAD, D + 2), dtype=ml_dtypes.bfloat16)
        xa[:, :D] = rows.astype(ml_dtypes.bfloat16)
        xa[:, D] = ml_dtypes.bfloat16(1.0)

        xt = np.ascontiguousarray(rows.T)

        sg = np.full(NPAD, 999.0, dtype=np.float32)
        loc = s2g[sel[vmask]].astype(np.int64) - m * SEG
        sg[vmask] = (loc % P).astype(np.float32)
        sg = sg.reshape(NT, P, 1)

        hgt = np.ascontiguousarray(h_graph[m * SEG : (m + 1) * SEG].T)

        im = dict(
            xt=xt, xa=xa, sg=sg, w1p=w1p_full, hgt=hgt, wc=wc_np,
            iota=iota_np, ident=ident_np,
        )
        if has_bc:
            im["bcb"] = bcb_np
        in_maps.append(im)

    meta = dict(
        B=B, N=N, D=D, H=H, T=T, SEG=SEG, NPAD=NPAD, CB=CB, NT=NT,
        PPOS=PPOS, b2f=b2f, has_b1=has_b1, has_bc=has_bc,
        slot_orig=slot_orig,
    )
    return meta, in_maps


def _finish(meta, results):
    """Gather shards and finish the O(B*D) contrastive-loss algebra."""
    B, N, T, SEG = meta["B"], meta["N"], meta["T"], meta["SEG"]
    slot_orig = meta["slot_orig"]

    out_full = np.concatenate([r["out_o"] for r in results], axis=0).astype(np.float32)
    aligned = np.concatenate([r["al_o"] for r in results], axis=0).astype(np.float64)
    env = np.concatenate([r["ev_o"] for r in results], axis=0).astype(np.float64)

    mask_full = np.empty(N, dtype=np.float32)
    for m in range(NCORES):
        vals = results[m]["mask_o"].reshape(-1)
        sel = slot_orig[m]
        vmask = sel >= 0
        mask_full[sel[vmask]] = vals[vmask]

    an = np.maximum(np.linalg.norm(aligned, axis=1), EPS)
    en = np.maximum(np.linalg.norm(env, axis=1), EPS)
    ahat = aligned / an[:, None]
    ehat = env / en[:, None]
    nz_pos = np.any(aligned != 0, axis=1)
    nz_neg = np.any(env != 0, axis=1)
    S_a = ahat.sum(axis=0)
    S_e = ehat.sum(axis=0)  # zero rows contribute zero, matching the nz mask
    pos_num = max(int(nz_pos.sum()) - 1, 1)
    num_env = int(nz_neg.sum())
    positive = (B - ahat @ S_a) / pos_num
    negative = (num_env - ahat @ S_e) / max(num_env, 1)
    contrib = np.maximum(positive - negative + 1.0, 0.0) * nz_pos * (num_env > 0)
    loss = np.float32(contrib.sum() / B)

    return out_full, loss, mask_full


_PROGRAM_CACHE = {}


def _run(inputs, timing_iters=0):
    meta, in_maps = _prepare(inputs)
    key = (meta["NPAD"], meta["CB"], meta["PPOS"], meta["b2f"],
           meta["has_b1"], meta["has_bc"], meta["T"])
    nc = _PROGRAM_CACHE.get(key)
    if nc is None:
        nc = _build_program(
            meta["NPAD"], meta["CB"], meta["PPOS"], meta["D"], meta["H"],
            meta["T"], meta["b2f"], meta["has_b1"], meta["has_bc"],
        )
        _PROGRAM_CACHE[key] = nc
    results, exec_ns = _pjrt_run(nc, in_maps, timing_iters=timing_iters)
    outputs = _finish(meta, results)
    return outputs, exec_ns


def _pjrt_run(nc, in_maps, timing_iters=0):
    """Mirror of bass2jax.run_bass_via_pjrt's multi-core path, without
    output-buffer donation (every output byte is written by the kernel) so
    the jitted executable can be re-invoked on device-resident inputs for
    timing."""
    import time

    import jax
    import jax.numpy as jnp
    from jax.experimental.shard_map import shard_map
    from jax.sharding import Mesh, PartitionSpec

    from concourse import bass2jax, mybir

    bass2jax.install_neuronx_cc_hook()
    n_cores = len(in_maps)

    partition_name = nc.partition_id_tensor.name if nc.partition_id_tensor else None
    in_names, out_names, out_avals = [], [], []
    for alloc in nc.m.functions[0].allocations:
        if not isinstance(alloc, mybir.MemoryLocationSet):
            continue
        name = alloc.memorylocations[0].name
        if alloc.kind == "ExternalInput":
            if name != partition_name:
                in_names.append(name)
        elif alloc.kind == "ExternalOutput":
            out_names.append(name)
            out_avals.append(
                jax.core.ShapedArray(tuple(alloc.tensor_shape), mybir.dt.np(alloc.dtype))
            )
    n_params = len(in_names)
    all_names = in_names + out_names
    if partition_name is not None:
        all_names.append(partition_name)

    def _body(*args):
        operands = list(args)
        if partition_name is not None:
            operands.append(bass2jax.partition_id_tensor())
        outs = bass2jax._bass_exec_p.bind(
            *operands,
            out_avals=tuple(out_avals),
            in_names=tuple(all_names),
            out_names=tuple(out_names),
            lowering_input_output_aliases=(),
            sim_require_finite=True,
            sim_require_nnan=True,
            nc=nc,
        )
        return tuple(outs)

    devices = jax.devices()[:n_cores]
    assert len(devices) == n_cores
    mesh = Mesh(np.asarray(devices), ("core",))
    in_specs = (PartitionSpec("core"),) * (n_params + len(out_names))
    out_specs = (PartitionSpec("core"),) * len(out_names)
    fn = jax.jit(
        shard_map(_body, mesh=mesh, in_specs=in_specs, out_specs=out_specs,
                  check_rep=False),
        keep_unused=True,
    )

    concat_in = [
        np.concatenate([np.asarray(in_maps[c][name]) for c in range(n_cores)], axis=0)
        for name in in_names
    ]
    concat_zeros = [
        np.zeros((n_cores * a.shape[0], *a.shape[1:]), a.dtype) for a in out_avals
    ]
    args_dev = [jax.device_put(x) for x in concat_in + concat_zeros]
    for a in args_dev:
        a.block_until_ready()

    out_arrs = fn(*args_dev)
    jax.block_until_ready(out_arrs)

    exec_ns = None
    if timing_iters > 0:
        times = []
        for _ in range(timing_iters):
            t0 = time.perf_counter()
            o = fn(*args_dev)
            jax.block_until_ready(o)
            times.append(time.perf_counter() - t0)
        exec_ns = int(min(times) * 1e9)

    results = [
        {
            name: np.asarray(out_arrs[i]).reshape(n_cores, *out_avals[i].shape)[c]
            for i, name in enumerate(out_names)
        }
        for c in range(n_cores)
    ]
    return results, exec_ns


def kernel(**inputs):
    outputs, _ = _run(inputs)
    return outputs
